# revision 38
# baseline (speedup 1.0000x reference)
"""DeepSeekMOE grouped masked GEMM kernel for 8 Trainium2 NeuronCores.

Expert-parallel: core g owns expert group g. Per core:
  out_ug = x_ug[g] @ w_ug[g].T   [32, 2816]
  out_dn = x_dn[g] @ w_dn[g].T   [32, 4096]
  rows >= masked_m[g] are zero (applied by zeroing x rows on host).
Output [8, 32, 6912] = concat(out_ug, out_dn) per group.

Memory-bound: the weights dominate HBM traffic and stream through each
core exactly once (~17.3 MB/core; measured local DMA rate ~305 GB/s is
byte-bound, so dtype width is the lever). They are cast to float8_e3m4
on host: the e3m4 mantissa (4 bits) keeps the quantization noise at
~1.35e-2 rel, inside the 2e-2 gate (e4m3 would fail at ~2.7e-2).
Weights are pre-scaled by 64 on host (randn*0.02 would land in e3m4's
subnormal range) and the inverse 1/64 is folded into the bf16
activations (exact, power of 2). Matmuls are bf16(x, stationary) x
fp8(w, moving) accumulating fp32 in PSUM over K; fp8 moving runs at
bf16 speed, which is fine since the PE (~30 us) hides under the DMA.
Outputs leave the device as bf16 (halves the out DMA) and are upcast
to fp32 on host.

DMA layout: both x and w are pre-packed on host to the exact SBUF
layout, so every DMA is a flat 2D copy of a contiguous DRAM block.
Weight slab s is one contiguous P*slen*N-byte extent whose partition-k
row is a single slen*N-byte run (A/B'd against [K,N]-rowwise and
[P, KC*N]-column-sliced sources: 60.9 / 111 us DMA-only; this layout
plus ramped slab sizes measures fastest). Slab sizes ramp up at the
start (PE starts ~1 us in) and down at the end (short post-last-DMA
tail), with chunk-wise PSUM eviction inline with the last k-chunk.
"""
import numpy as np

import concourse.bass as bass
import concourse.bacc as bacc
import concourse.mybir as mybir
import concourse.tile as tile
from concourse.bass_utils import run_bass_kernel_spmd

G, M = 8, 32
K_UG, N_UG = 4096, 2816
K_DN, N_DN = 1408, 4096
N_OUT = N_UG + N_DN
P = 128
KC_UG = K_UG // P  # 32 k-chunks
KC_DN = K_DN // P  # 11 k-chunks
SLAB = 4  # k-chunks per weight DMA
WBUFS = 4

f32 = mybir.dt.float32
bf16 = mybir.dt.bfloat16
f8 = mybir.dt.float8e3
W_SCALE = 64.0  # host-side weight prescale (power of 2; inverse folded into x)
F8_MAX = 15.5  # e3m4 max normal

TRACE = False  # NTFF tracing unavailable over axon; timing lives in bench.py
_cache = {}


def _np_bf16():
    import ml_dtypes

    return np.dtype(ml_dtypes.bfloat16)


def _np_f8e3():
    import ml_dtypes

    return np.dtype(ml_dtypes.float8_e3m4)


def _n_chunks(n_total):
    chunks = []
    n0 = 0
    while n0 < n_total:
        nlen = min(512, n_total - n0)
        chunks.append((n0, nlen))
        n0 += nlen
    return chunks


def _slabs_of(kc_total, slab):
    slabs = []
    c0 = 0
    while c0 < kc_total:
        slen = min(slab, kc_total - c0)
        slabs.append((c0, slen))
        c0 += slen
    return slabs


def _slab_plan(kc_tot, slab, mi):
    """Slab partition for matmul mi (0=ug first, 1=dn last).

    Under the default slab size the first type ramps up (tiny first slabs
    so the PE starts ~1us in instead of waiting for a 1.4MB DMA) and the
    last type ramps down (tiny final slab so the post-last-DMA PE tail is
    short). Probes with a non-default slab get the plain partition.
    """
    if slab != SLAB:
        return _slabs_of(kc_tot, slab)
    if mi == 0:
        rest = kc_tot - 4
        lens = [1, 1, 2] + [SLAB] * (rest // SLAB)
        if rest % SLAB:
            lens.append(rest % SLAB)
    else:
        rest = kc_tot - 3
        lens = [SLAB] * (rest // SLAB) + [2, 1]
        if rest % SLAB:
            lens.insert(0, rest % SLAB)
    if sum(lens) != kc_tot:  # fallback for unexpected kc_tot
        return _slabs_of(kc_tot, slab)
    plan = []
    c0 = 0
    for ln in lens:
        plan.append((c0, ln))
        c0 += ln
    return plan


def _build_program(reps=1, n_stride=1, dma_frac=1, slab=SLAB, wbufs=WBUFS,
                   alt_engine=False, no_dma=False, no_pe=False,
                   io_scalar=True, quad=True, no_io=False,
                   io_gpsimd=False, split_dma=False, dual_queue=False,
                   w_bf16=False, tri_queue=False):
    """reps>1 wraps the body in a HW loop — bench-only, for slope timing.

    n_stride/dma_frac are bench-only probes: compute every n_stride-th n-chunk
    (cuts PE work) / load only 1/dma_frac of each weight slab (cuts DMA).

    quad=True runs the PE in 128x32 column-tiled mode: the N range is split
    across 2 independent column tiles (tile_position col = 0/64, each
    accumulating into its own PSUM partition quadrant), doubling moving-
    operand ingest since M=32 uses only a quarter of the array's columns.
    (4-way would need PSUM base partition 96, which the AP cannot encode.)
    """
    nc = bacc.Bacc("TRN2", target_bir_lowering=False, debug=False)

    wdt = bf16 if w_bf16 else f8
    xp_ug = nc.dram_tensor("xp_ug", [P, KC_UG * M], bf16, kind="ExternalInput")
    wt_ug = nc.dram_tensor("wt_ug", [KC_UG * P * N_UG], wdt, kind="ExternalInput")
    xp_dn = nc.dram_tensor("xp_dn", [P, KC_DN * M], bf16, kind="ExternalInput")
    wt_dn = nc.dram_tensor("wt_dn", [KC_DN * P * N_DN], wdt, kind="ExternalInput")
    out = nc.dram_tensor("out", [M, N_OUT], bf16, kind="ExternalOutput")

    import contextlib

    with contextlib.ExitStack() as stack:
        tc = stack.enter_context(tile.TileContext(nc))
        wpool = stack.enter_context(tc.tile_pool(name="w", bufs=wbufs))
        misc = stack.enter_context(tc.tile_pool(name="misc", bufs=1))
        psum = stack.enter_context(tc.tile_pool(name="psum", bufs=1, space="PSUM"))
        if reps > 1:
            stack.enter_context(tc.For_i(0, reps, 1))
        if True:
            # Stationary activations, pre-packed on host to [128, KC*M]
            # (chunk c at columns [c*M, (c+1)*M)) so this DMA is contiguous.
            io_eng = nc.gpsimd if io_gpsimd else (
                nc.scalar if io_scalar else nc.sync
            )
            xug_t = misc.tile([P, KC_UG * M], bf16, tag="xug")
            io_eng.dma_start(xug_t[:], xp_ug[:])
            xdn_t = misc.tile([P, KC_DN * M], bf16, tag="xdn")
            io_eng.dma_start(xdn_t[:], xp_dn[:])

            slab_pair = (
                tuple(slab) if isinstance(slab, (tuple, list)) else (slab, slab)
            )
            # dn first, ug last: the final type's eviction is on the serial
            # tail, and ug's is 30% smaller ([32,2816] vs [32,4096]).
            for mi, (wt_d, xt_t, n_tot, kc_tot, out_off, oname) in enumerate((
                (wt_dn, xdn_t, N_DN, KC_DN, N_UG, "odn"),
                (wt_ug, xug_t, N_UG, KC_UG, 0, "oug"),
            )):
                nquads = 2 if quad else 1
                nq = n_tot // nquads
                qstep = 64  # AP base-partition encoding allows only 0/32/64
                nch = _n_chunks(nq)
                acc = (
                    None
                    if no_pe
                    else psum.tile(
                        [P if quad else M, nq], f32, tag="acc", bufs=2
                    )
                )
                # DRAM weights are pre-packed per slab as k-major blocks:
                # slab s is ONE contiguous DRAM extent of P*slen*N bytes,
                # inside which partition k's data (chunk-major, then n) is
                # a single slen*N-byte contiguous run. The slab DMA is a
                # flat 2D copy: block-contiguous on the DRAM side, long
                # runs on the SBUF side.
                o_t = (
                    None
                    if (no_pe or no_io)
                    else misc.tile([P if quad else M, nq], bf16, tag=oname)
                )
                for si, (c0, slen) in enumerate(
                    _slab_plan(kc_tot, slab_pair[mi], mi)
                ):
                    w_t = wpool.tile([P, slen * n_tot], wdt, tag="w")
                    if tri_queue:
                        eng = (nc.sync, nc.scalar, nc.gpsimd)[si % 3]
                    else:
                        eng = nc.scalar if (alt_engine and si % 2) else nc.sync
                    nload = n_tot // dma_frac
                    w_src = wt_d[
                        c0 * P * n_tot : (c0 + slen) * P * n_tot
                    ].rearrange("(k x) -> k x", k=P)
                    if not no_dma:
                        if split_dma and quad and dma_frac == 1:
                            # one DMA per PE column tile, on parallel HWDGE
                            # queues: each tile's matmuls depend only on
                            # their own half of the slab.
                            w_dst = w_t[:].rearrange(
                                "k (c n) -> k c n", c=slen
                            )
                            w_src3 = w_src.rearrange(
                                "k (c n) -> k c n", c=slen
                            )
                            for q2, eng2 in ((0, nc.sync), (1, nc.scalar)):
                                eng2.dma_start(
                                    w_dst[:, :, q2 * nq : (q2 + 1) * nq],
                                    w_src3[:, :, q2 * nq : (q2 + 1) * nq],
                                )
                        elif dual_queue and slen > 1 and dma_frac == 1:
                            # split the slab by k-chunk halves across both
                            # HWDGE queues; each half stays a flat block-
                            # contiguous 2D copy with (slen/2)*N-byte runs.
                            h = (slen // 2) * n_tot
                            nc.sync.dma_start(w_t[:, :h], w_src[:, :h])
                            nc.scalar.dma_start(
                                w_t[:, h : slen * n_tot],
                                w_src[:, h : slen * n_tot],
                            )
                        else:
                            eng.dma_start(
                                w_t[:, : slen * nload],
                                w_src[:, : slen * nload],
                            )
                    for c in range(slen):
                        kc = c0 + c
                        if no_pe:
                            continue
                        last_kc = kc == kc_tot - 1
                        for q in range(nquads):
                            lo = q * qstep
                            for ni, (n0, nlen) in enumerate(nch):
                                if ni % n_stride:
                                    continue
                                w0 = c * n_tot + q * nq + n0
                                nc.tensor.matmul(
                                    acc[lo : lo + M, n0 : n0 + nlen],
                                    xt_t[:, bass.ts(kc, M)],
                                    w_t[:, w0 : w0 + nlen],
                                    start=(kc == 0),
                                    stop=last_kc,
                                )
                                if last_kc and not no_io:
                                    # evict + store this n-chunk as soon as
                                    # its accumulation group closes, so the
                                    # tail after the final matmul is one
                                    # chunk deep instead of a full-width
                                    # evict + one big out DMA. Copies
                                    # alternate vector/scalar so both
                                    # engines share every quadrant's tail.
                                    if (q * len(nch) + ni) % 2 == 0 or split_dma:
                                        nc.vector.tensor_copy(
                                            o_t[lo : lo + M, n0 : n0 + nlen],
                                            acc[lo : lo + M, n0 : n0 + nlen],
                                        )
                                    else:
                                        nc.scalar.copy(
                                            o_t[lo : lo + M, n0 : n0 + nlen],
                                            acc[lo : lo + M, n0 : n0 + nlen],
                                        )
                                    io_eng.dma_start(
                                        out[
                                            :,
                                            out_off
                                            + q * nq
                                            + n0 : out_off
                                            + q * nq
                                            + n0
                                            + nlen,
                                        ],
                                        o_t[lo : lo + M, n0 : n0 + nlen],
                                    )
                if no_pe or no_io:
                    # keep `out` written so the verifier sees a writer
                    nc.gpsimd.dma_start(
                        out[:, out_off : out_off + KC_DN * M],
                        xt_t[:M, : KC_DN * M],
                    )
                    continue

    nc.compile()
    return nc


def _pack_x(x, kc):
    # [M, K] -> [P, kc*M] with chunk c at columns [c*M, (c+1)*M):
    # xp[k, c*M + m] = x[m, c*P + k]
    return np.ascontiguousarray(
        x.reshape(M, kc, P).transpose(2, 1, 0).reshape(P, kc * M)
    )


def _pack_w(w, kc, n, mi, slab=SLAB, w_bf16=False):
    # [N, K] -> flat [kc*P*n], slab by slab (same plan as the program).
    # Slab s (chunks c0..c0+slen) is a contiguous block laid out
    # [P, slen*n]: block[k, c*n + nn] = w[nn, (c0+c)*P + k]
    if w_bf16:
        w8 = (w * np.float32(W_SCALE)).astype(_np_bf16())
    else:
        f8np = _np_f8e3()
        w8 = np.clip(w * np.float32(W_SCALE), -F8_MAX, F8_MAX).astype(f8np)
    a = w8.reshape(n, kc, P).transpose(1, 2, 0)  # [kc, P, n]
    parts = []
    for c0, slen in _slab_plan(kc, slab, mi):
        parts.append(
            a[c0 : c0 + slen].transpose(1, 0, 2).reshape(P * slen * n)
        )
    return np.ascontiguousarray(np.concatenate(parts))


def prepare_in_maps(x_ug, w_ug, x_dn, w_dn, masked_m, slab=SLAB, w_bf16=False):
    bf = _np_bf16()
    x_ug = np.asarray(x_ug, dtype=np.float32)
    w_ug = np.asarray(w_ug, dtype=np.float32)
    x_dn = np.asarray(x_dn, dtype=np.float32)
    w_dn = np.asarray(w_dn, dtype=np.float32)
    masked_m = np.asarray(masked_m)

    inv_s = np.float32(1.0 / W_SCALE)
    row = np.arange(M)
    in_maps = []
    for g in range(G):
        valid = (row < int(masked_m[g])).astype(np.float32)[:, None]
        in_maps.append(
            {
                "xp_ug": _pack_x((x_ug[g] * valid * inv_s).astype(bf), KC_UG),
                "wt_ug": _pack_w(w_ug[g], KC_UG, N_UG, 1, slab, w_bf16),
                "xp_dn": _pack_x((x_dn[g] * valid * inv_s).astype(bf), KC_DN),
                "wt_dn": _pack_w(w_dn[g], KC_DN, N_DN, 0, slab, w_bf16),
            }
        )
    return in_maps


def kernel(x_ug, w_ug, x_dn, w_dn, masked_m):
    if "nc" not in _cache:
        _cache["nc"] = _build_program()
    nc = _cache["nc"]

    in_maps = prepare_in_maps(x_ug, w_ug, x_dn, w_dn, masked_m)

    res = None
    for attempt in range(3):
        try:
            res = run_bass_kernel_spmd(
                nc, in_maps, core_ids=list(range(G)), trace=TRACE
            )
            break
        except Exception:
            if attempt == 2:
                raise
            # Transient NRT/device failures: reset jax backends and retry.
            import time

            try:
                import jax

                jax.clear_caches()
                import jax.extend.backend as _jb

                _jb.clear_backends()
            except Exception:
                pass
            time.sleep(20.0 * (attempt + 1))
    if TRACE:
        _cache["last_result"] = res
    return np.stack(
        [res.results[g]["out"].astype(np.float32) for g in range(G)], axis=0
    )



# revision 44
# speedup vs baseline: 1.1152x; 1.1152x over previous
"""DeepSeekMOE grouped masked GEMM kernel for 8 Trainium2 NeuronCores.

Expert-parallel: core g owns expert group g. Per core:
  out_ug = x_ug[g] @ w_ug[g].T   [32, 2816]
  out_dn = x_dn[g] @ w_dn[g].T   [32, 4096]
  rows >= masked_m[g] are zero (applied by zeroing x rows on host).
Output [8, 32, 6912] = concat(out_ug, out_dn) per group.

Memory-bound: the weights dominate HBM traffic and stream through each
core exactly once (~17.3 MB/core; measured local DMA rate ~305 GB/s is
byte-bound, so dtype width is the lever). They are cast to float8_e3m4
on host: the e3m4 mantissa (4 bits) keeps the quantization noise at
~1.35e-2 rel, inside the 2e-2 gate (e4m3 would fail at ~2.7e-2).
Weights are pre-scaled by 64 on host (randn*0.02 would land in e3m4's
subnormal range) and the inverse 1/64 is folded into the bf16
activations (exact, power of 2). Matmuls are bf16(x, stationary) x
fp8(w, moving) accumulating fp32 in PSUM over K; fp8 moving runs at
bf16 speed, which is fine since the PE (~30 us) hides under the DMA.
Outputs leave the device as bf16 (halves the out DMA) and are upcast
to fp32 on host.

DMA layout: both x and w are pre-packed on host to the exact SBUF
layout, so every DMA is a flat 2D copy of a contiguous DRAM block.
Weight slab s is one contiguous P*slen*N-byte extent whose partition-k
row is a single slen*N-byte run (A/B'd against [K,N]-rowwise and
[P, KC*N]-column-sliced sources: 60.9 / 111 us DMA-only; this layout
plus ramped slab sizes measures fastest). Slab sizes ramp up at the
start (PE starts ~1 us in) and down at the end (short post-last-DMA
tail), with chunk-wise PSUM eviction inline with the last k-chunk.
"""
import numpy as np

import concourse.bass as bass
import concourse.bacc as bacc
import concourse.mybir as mybir
import concourse.tile as tile
from concourse.bass_utils import run_bass_kernel_spmd

G, M = 8, 32
K_UG, N_UG = 4096, 2816
K_DN, N_DN = 1408, 4096
N_OUT = N_UG + N_DN
P = 128
KC_UG = K_UG // P  # 32 k-chunks
KC_DN = K_DN // P  # 11 k-chunks
SLAB = 4  # k-chunks per weight DMA
WBUFS = 4

f32 = mybir.dt.float32
bf16 = mybir.dt.bfloat16
f8 = mybir.dt.float8e3
W_SCALE = 64.0  # host-side weight prescale (power of 2; inverse folded into x)
F8_MAX = 15.5  # e3m4 max normal

TRACE = False  # NTFF tracing unavailable over axon; timing lives in bench.py
_cache = {}


def _np_bf16():
    import ml_dtypes

    return np.dtype(ml_dtypes.bfloat16)


def _np_f8e3():
    import ml_dtypes

    return np.dtype(ml_dtypes.float8_e3m4)


def _n_chunks(n_total):
    chunks = []
    n0 = 0
    while n0 < n_total:
        nlen = min(512, n_total - n0)
        chunks.append((n0, nlen))
        n0 += nlen
    return chunks


def _slabs_of(kc_total, slab):
    slabs = []
    c0 = 0
    while c0 < kc_total:
        slen = min(slab, kc_total - c0)
        slabs.append((c0, slen))
        c0 += slen
    return slabs


def _slab_plan(kc_tot, slab, mi):
    """Slab partition for matmul mi (0=ug first, 1=dn last).

    Under the default slab size the first type ramps up (tiny first slabs
    so the PE starts ~1us in instead of waiting for a 1.4MB DMA) and the
    last type ramps down (tiny final slab so the post-last-DMA PE tail is
    short). Probes with a non-default slab get the plain partition.
    """
    if slab != SLAB:
        return _slabs_of(kc_tot, slab)
    if mi == 0:
        lens = [1, 1, 2] + [SLAB] * ((kc_tot - 4) // SLAB)
    else:
        lens = [SLAB] * ((kc_tot - 3) // SLAB) + [2, 1]
    if sum(lens) != kc_tot:  # fallback for unexpected kc_tot
        return _slabs_of(kc_tot, slab)
    plan = []
    c0 = 0
    for ln in lens:
        plan.append((c0, ln))
        c0 += ln
    return plan


def _build_program(reps=1, n_stride=1, dma_frac=1, slab=SLAB, wbufs=WBUFS,
                   alt_engine=False, no_dma=False, no_pe=False,
                   io_scalar=True, quad=True, no_io=False,
                   io_gpsimd=False, split_dma=False, dual_queue=False,
                   w_bf16=False, tri_queue=False):
    """reps>1 wraps the body in a HW loop — bench-only, for slope timing.

    n_stride/dma_frac are bench-only probes: compute every n_stride-th n-chunk
    (cuts PE work) / load only 1/dma_frac of each weight slab (cuts DMA).

    quad=True runs the PE in 128x32 column-tiled mode: the N range is split
    across 2 independent column tiles (tile_position col = 0/64, each
    accumulating into its own PSUM partition quadrant), doubling moving-
    operand ingest since M=32 uses only a quarter of the array's columns.
    (4-way would need PSUM base partition 96, which the AP cannot encode.)
    """
    nc = bacc.Bacc("TRN2", target_bir_lowering=False, debug=False)

    wdt = bf16 if w_bf16 else f8
    xp_ug = nc.dram_tensor("xp_ug", [P, KC_UG * M], bf16, kind="ExternalInput")
    wt_ug = nc.dram_tensor("wt_ug", [KC_UG * P * N_UG], wdt, kind="ExternalInput")
    xp_dn = nc.dram_tensor("xp_dn", [P, KC_DN * M], bf16, kind="ExternalInput")
    wt_dn = nc.dram_tensor("wt_dn", [KC_DN * P * N_DN], wdt, kind="ExternalInput")
    out = nc.dram_tensor("out", [M, N_OUT], bf16, kind="ExternalOutput")

    import contextlib

    with contextlib.ExitStack() as stack:
        tc = stack.enter_context(tile.TileContext(nc))
        wpool = stack.enter_context(tc.tile_pool(name="w", bufs=wbufs))
        misc = stack.enter_context(tc.tile_pool(name="misc", bufs=1))
        psum = stack.enter_context(tc.tile_pool(name="psum", bufs=1, space="PSUM"))
        if reps > 1:
            stack.enter_context(tc.For_i(0, reps, 1))
        if True:
            # Stationary activations, pre-packed on host to [128, KC*M]
            # (chunk c at columns [c*M, (c+1)*M)) so this DMA is contiguous.
            io_eng = nc.gpsimd if io_gpsimd else (
                nc.scalar if io_scalar else nc.sync
            )
            xug_t = misc.tile([P, KC_UG * M], bf16, tag="xug")
            io_eng.dma_start(xug_t[:], xp_ug[:])
            xdn_t = misc.tile([P, KC_DN * M], bf16, tag="xdn")
            io_eng.dma_start(xdn_t[:], xp_dn[:])

            slab_pair = (
                tuple(slab) if isinstance(slab, (tuple, list)) else (slab, slab)
            )
            for mi, (wt_d, xt_t, n_tot, kc_tot, out_off, oname) in enumerate((
                (wt_ug, xug_t, N_UG, KC_UG, 0, "oug"),
                (wt_dn, xdn_t, N_DN, KC_DN, N_UG, "odn"),
            )):
                nquads = 2 if quad else 1
                nq = n_tot // nquads
                qstep = 64  # AP base-partition encoding allows only 0/32/64
                nch = _n_chunks(nq)
                acc = (
                    None
                    if no_pe
                    else psum.tile(
                        [P if quad else M, nq], f32, tag="acc", bufs=2
                    )
                )
                # DRAM weights are pre-packed per slab as k-major blocks:
                # slab s is ONE contiguous DRAM extent of P*slen*N bytes,
                # inside which partition k's data (chunk-major, then n) is
                # a single slen*N-byte contiguous run. The slab DMA is a
                # flat 2D copy: block-contiguous on the DRAM side, long
                # runs on the SBUF side.
                o_t = (
                    None
                    if (no_pe or no_io)
                    else misc.tile([P if quad else M, nq], bf16, tag=oname)
                )
                for si, (c0, slen) in enumerate(
                    _slab_plan(kc_tot, slab_pair[mi], mi)
                ):
                    w_t = wpool.tile([P, slen * n_tot], wdt, tag="w")
                    if tri_queue:
                        eng = (nc.sync, nc.scalar, nc.gpsimd)[si % 3]
                    else:
                        eng = nc.scalar if (alt_engine and si % 2) else nc.sync
                    nload = n_tot // dma_frac
                    w_src = wt_d[
                        c0 * P * n_tot : (c0 + slen) * P * n_tot
                    ].rearrange("(k x) -> k x", k=P)
                    if not no_dma:
                        if split_dma and quad and dma_frac == 1:
                            # one DMA per PE column tile, on parallel HWDGE
                            # queues: each tile's matmuls depend only on
                            # their own half of the slab.
                            w_dst = w_t[:].rearrange(
                                "k (c n) -> k c n", c=slen
                            )
                            w_src3 = w_src.rearrange(
                                "k (c n) -> k c n", c=slen
                            )
                            for q2, eng2 in ((0, nc.sync), (1, nc.scalar)):
                                eng2.dma_start(
                                    w_dst[:, :, q2 * nq : (q2 + 1) * nq],
                                    w_src3[:, :, q2 * nq : (q2 + 1) * nq],
                                )
                        elif dual_queue and slen > 1 and dma_frac == 1:
                            # split the slab by k-chunk halves across both
                            # HWDGE queues; each half stays a flat block-
                            # contiguous 2D copy with (slen/2)*N-byte runs.
                            h = (slen // 2) * n_tot
                            nc.sync.dma_start(w_t[:, :h], w_src[:, :h])
                            nc.scalar.dma_start(
                                w_t[:, h : slen * n_tot],
                                w_src[:, h : slen * n_tot],
                            )
                        elif quad and dma_frac == 1:
                            # one flat block-contiguous DMA per PE column
                            # tile (quad-major packed), so each tile's
                            # matmuls gate on their own half of the slab
                            # instead of the whole slab DMA.
                            hq = slen * nq
                            for q2 in range(nquads):
                                base = c0 * P * n_tot + q2 * P * hq
                                eng.dma_start(
                                    w_t[:, q2 * hq : (q2 + 1) * hq],
                                    wt_d[
                                        base : base + P * hq
                                    ].rearrange("(k x) -> k x", k=P),
                                )
                        else:
                            eng.dma_start(
                                w_t[:, : slen * nload],
                                w_src[:, : slen * nload],
                            )
                    for c in range(slen):
                        kc = c0 + c
                        if no_pe:
                            continue
                        last_kc = kc == kc_tot - 1
                        for q in range(nquads):
                            lo = q * qstep
                            for ni, (n0, nlen) in enumerate(nch):
                                if ni % n_stride:
                                    continue
                                w0 = (
                                    (q * slen + c) * nq + n0
                                    if quad
                                    else c * n_tot + n0
                                )
                                nc.tensor.matmul(
                                    acc[lo : lo + M, n0 : n0 + nlen],
                                    xt_t[:, bass.ts(kc, M)],
                                    w_t[:, w0 : w0 + nlen],
                                    start=(kc == 0),
                                    stop=last_kc,
                                )
                                if last_kc and not no_io:
                                    # evict + store this n-chunk as soon as
                                    # its accumulation group closes, so the
                                    # tail after the final matmul is one
                                    # chunk deep instead of a full-width
                                    # evict + one big out DMA.
                                    if q == 0 or split_dma:
                                        nc.vector.tensor_copy(
                                            o_t[lo : lo + M, n0 : n0 + nlen],
                                            acc[lo : lo + M, n0 : n0 + nlen],
                                        )
                                    else:
                                        nc.scalar.copy(
                                            o_t[lo : lo + M, n0 : n0 + nlen],
                                            acc[lo : lo + M, n0 : n0 + nlen],
                                        )
                                    io_eng.dma_start(
                                        out[
                                            :,
                                            out_off
                                            + q * nq
                                            + n0 : out_off
                                            + q * nq
                                            + n0
                                            + nlen,
                                        ],
                                        o_t[lo : lo + M, n0 : n0 + nlen],
                                    )
                if no_pe or no_io:
                    # keep `out` written so the verifier sees a writer
                    nc.gpsimd.dma_start(
                        out[:, out_off : out_off + KC_DN * M],
                        xt_t[:M, : KC_DN * M],
                    )
                    continue

    nc.compile()
    return nc


def _pack_x(x, kc):
    # [M, K] -> [P, kc*M] with chunk c at columns [c*M, (c+1)*M):
    # xp[k, c*M + m] = x[m, c*P + k]
    return np.ascontiguousarray(
        x.reshape(M, kc, P).transpose(2, 1, 0).reshape(P, kc * M)
    )


def _pack_w(w, kc, n, mi, slab=SLAB, w_bf16=False):
    # [N, K] -> flat [kc*P*n], slab by slab (same plan as the program).
    # Slab s (chunks c0..c0+slen) is two contiguous per-quadrant blocks,
    # each laid out [P, slen*nq] (quad q owns n columns [q*nq,(q+1)*nq)):
    #   block_q[k, c*nq + nn] = w[q*nq + nn, (c0+c)*P + k]
    # so each quadrant's DMA is one flat block-contiguous copy and its
    # column tile's matmuls gate only on their own half of the slab.
    if w_bf16:
        w8 = (w * np.float32(W_SCALE)).astype(_np_bf16())
    else:
        f8np = _np_f8e3()
        w8 = np.clip(w * np.float32(W_SCALE), -F8_MAX, F8_MAX).astype(f8np)
    nq = n // 2
    a = w8.reshape(n, kc, P).transpose(1, 2, 0)  # [kc, P, n]
    parts = []
    for c0, slen in _slab_plan(kc, slab, mi):
        for q in range(2):
            parts.append(
                a[c0 : c0 + slen, :, q * nq : (q + 1) * nq]
                .transpose(1, 0, 2)
                .reshape(P * slen * nq)
            )
    return np.ascontiguousarray(np.concatenate(parts))


def prepare_in_maps(x_ug, w_ug, x_dn, w_dn, masked_m, slab=SLAB, w_bf16=False):
    bf = _np_bf16()
    x_ug = np.asarray(x_ug, dtype=np.float32)
    w_ug = np.asarray(w_ug, dtype=np.float32)
    x_dn = np.asarray(x_dn, dtype=np.float32)
    w_dn = np.asarray(w_dn, dtype=np.float32)
    masked_m = np.asarray(masked_m)

    inv_s = np.float32(1.0 / W_SCALE)
    row = np.arange(M)
    in_maps = []
    for g in range(G):
        valid = (row < int(masked_m[g])).astype(np.float32)[:, None]
        in_maps.append(
            {
                "xp_ug": _pack_x((x_ug[g] * valid * inv_s).astype(bf), KC_UG),
                "wt_ug": _pack_w(w_ug[g], KC_UG, N_UG, 0, slab, w_bf16),
                "xp_dn": _pack_x((x_dn[g] * valid * inv_s).astype(bf), KC_DN),
                "wt_dn": _pack_w(w_dn[g], KC_DN, N_DN, 1, slab, w_bf16),
            }
        )
    return in_maps


def kernel(x_ug, w_ug, x_dn, w_dn, masked_m):
    if "nc" not in _cache:
        _cache["nc"] = _build_program()
    nc = _cache["nc"]

    in_maps = prepare_in_maps(x_ug, w_ug, x_dn, w_dn, masked_m)

    res = None
    for attempt in range(3):
        try:
            res = run_bass_kernel_spmd(
                nc, in_maps, core_ids=list(range(G)), trace=TRACE
            )
            break
        except Exception:
            if attempt == 2:
                raise
            # Transient NRT/device failures: reset jax backends and retry.
            import time

            try:
                import jax

                jax.clear_caches()
                import jax.extend.backend as _jb

                _jb.clear_backends()
            except Exception:
                pass
            time.sleep(20.0 * (attempt + 1))
    if TRACE:
        _cache["last_result"] = res
    return np.stack(
        [res.results[g]["out"].astype(np.float32) for g in range(G)], axis=0
    )

